# revision 33
# baseline (speedup 1.0000x reference)
"""Trainium2 Bass kernel for nn_AttentionalGNN (self-contained).

  xs/xt = standardize(p_src/p_tar).T ; ds/dt = mlp_dis(standardize(dis).T)
  delta0 = attn(xs, xt, xt); delta1 = attn(xt, xs, xs)
  ps = delta0*xt; pt = delta1*xs
  delta0' = attn(ds, dt, ps); delta1' = attn(dt, ds, pt)
  out_s = xs + mlp(cat(xs, delta0')); out_t likewise
  return ||mean_n(out_s) - mean_n(out_t)||^2

8-core SPMD: scale stats replicated; queries sharded 512/core for all four
attention calls (keys/values replicated); mlp_dis sharded over N with BN-stat
AllReduce + AllGather; round-1 deltas AllGathered (bf16) for the gating
products; final MLP sharded over N with AllReduced BN stats; final column
means AllReduced; the scalar is computed on every core.

Attention uses transposed scores (keys on partitions, queries on free) so no
transposes are needed anywhere: scoresT = K_h^T Q_h via one K=64 matmul per
key m-tile; exp on ScalarE (scale=1/8, no max subtraction - scores are O(10));
softmax denominator comes from a ones column prepended to V^T inside the PV
matmul (out partition 0 = denom, 1..64 = data); per-head normalization happens
at PSUM evacuation with a PE-broadcast reciprocal; the merge accumulates per
head with K=64 matmuls so everything stays partition-base-aligned. Head
channels are permuted host-side (d*4+h -> h*64+d) so head slices are
contiguous; merge weight columns permuted to match; V bias folded into the
merge bias (bm_eff = am_b + am_w @ av_b).
"""

import numpy as np
import ml_dtypes

D, H, HD, S, N, EPS = 256, 4, 64, 128, 4096, 1e-5
NC = 8
NQ = N // NC            # 512 queries per core
MT = N // 128           # 32 key m-tiles
HB = HD + 1             # per-head V^T block: [ones | V] = 65 cols

_CACHE = {}


def _build_program(dbg=False):
    import contextlib
    import concourse.bass as bass
    import concourse.bacc as bacc
    import concourse.tile as tile
    import concourse.mybir as mybir

    FP32 = mybir.dt.float32
    BF16 = mybir.dt.bfloat16
    AF = mybir.ActivationFunctionType
    ALU = mybir.AluOpType
    AX = mybir.AxisListType

    nc = bacc.Bacc(
        "TRN2",
        target_bir_lowering=False,
        debug=False,
        enable_asserts=False,
        num_devices=NC,
    )

    def din(name, shape, dt):
        return nc.dram_tensor(name, shape, dt, kind="ExternalInput").ap()

    psT = din("psT", [D, N], FP32)
    ptT = din("ptT", [D, N], FP32)
    dsT = din("dsT", [S, N], FP32)
    dtT = din("dtT", [S, N], FP32)
    ops = din("ops", [D, NQ], FP32)
    opt_ = din("opt", [D, NQ], FP32)
    ods = din("ods", [S, NQ], FP32)
    odt = din("odt", [S, NQ], FP32)
    wqT = din("wqT", [D, D], BF16)
    wkT = din("wkT", [D, D], BF16)
    wvT = din("wvT", [D, D], BF16)
    wmT = din("wmT", [D, D], BF16)
    bqv = din("bq", [D, 1], FP32)
    bkv = din("bk", [D, 1], FP32)
    bmv = din("bm", [D, 1], FP32)
    m1T = din("m1T", [2 * D, 2 * D], BF16)
    m1b = din("m1b", [2 * D, 1], FP32)
    m1g = din("m1g", [2 * D, 1], FP32)
    m1be = din("m1be", [2 * D, 1], FP32)
    m2T = din("m2T", [2 * D, D], BF16)
    m2b = din("m2b", [D, 1], FP32)
    d1T = din("d1T", [S, D], BF16)
    d1b = din("d1b", [D, 1], FP32)
    d1g = din("d1g", [D, 1], FP32)
    d1be = din("d1be", [D, 1], FP32)
    d2T = din("d2T", [D, D], BF16)
    d2b = din("d2b", [D, 1], FP32)
    out_dram = nc.dram_tensor("out", [1, 1], FP32, kind="ExternalOutput").ap()

    RG = [list(range(NC))]
    _dbg_done = set()

    def mkdbg(dma_fn, name, src_ap, shape, dt):
        if not dbg or name in _dbg_done:
            return
        _dbg_done.add(name)
        d = nc.dram_tensor(name, list(shape), dt, kind="ExternalOutput").ap()
        dma_fn(d[tuple(slice(None) for _ in shape)], src_ap)

    with tile.TileContext(nc) as tc:
        st = contextlib.ExitStack()
        PA = st.enter_context(tc.tile_pool(name="persistA", bufs=1))
        Ppr = st.enter_context(tc.tile_pool(name="probs", bufs=4))
        Psc = st.enter_context(
            tc.tile_pool(name="psum_sc", bufs=3, space=bass.MemorySpace.PSUM))
        Pout = st.enter_context(
            tc.tile_pool(name="psum_out", bufs=1, space=bass.MemorySpace.PSUM))
        Pbc = st.enter_context(
            tc.tile_pool(name="psum_bc", bufs=1, space=bass.MemorySpace.PSUM))
        Dram = st.enter_context(tc.tile_pool(name="dram", bufs=1, space="DRAM"))

        def pa(name, shape, dt, tag=None):
            return PA.tile(shape, dt, name=name, tag=tag or name)

        # --- persistA: needed from preprocessing onward ---
        xs_bf = pa("xs_bf", [128, 2, 8, 512], BF16)
        xt_bf = pa("xt_bf", [128, 2, 8, 512], BF16)
        xs_own = pa("xs_own", [128, 2, 512], FP32)
        xt_own = pa("xt_own", [128, 2, 512], FP32)
        xs_own_bf = pa("xs_own_bf", [128, 2, 512], BF16)
        xt_own_bf = pa("xt_own_bf", [128, 2, 512], BF16)
        ds_own_bf = pa("ds_own_bf", [128, 2, 512], BF16)
        dt_own_bf = pa("dt_own_bf", [128, 2, 512], BF16)
        x1d_s = pa("x1d_s", [128, 2, 512], FP32)
        x1d_t = pa("x1d_t", [128, 2, 512], FP32)
        sc_ds = pa("sc_ds", [128, 512], BF16)
        sc_dt = pa("sc_dt", [128, 512], BF16)
        r_d = pa("r_d", [128, 2, 512], BF16)
        Qb = pa("Qb", [128, 2, 512], BF16)
        An = pa("An", [64, 4, 512], BF16)       # attn out per head (raw->normed)
        d_bf = pa("d_bf", [128, 2, 512], BF16)  # round-1 delta0 own
        d1own_bf = pa("d1own_bf", [128, 2, 512], BF16)
        dp0_f = pa("dp0_f", [128, 2, 512], FP32)
        dp1_f = pa("dp1_f", [128, 2, 512], FP32)
        dp0_bf = pa("dp0_bf", [128, 2, 512], BF16)
        dp1_bf = pa("dp1_bf", [128, 2, 512], BF16)
        ones = pa("ones", [128, 64], FP32)
        wq_s = pa("wq_s", [128, 2, 256], BF16)
        wk_s = pa("wk_s", [128, 2, 256], BF16)
        wv_s = pa("wv_s", [128, 2, 256], BF16)
        wm_r = pa("wm_r", [64, 4, 256], BF16)   # head-major merge weights
        m1_s = pa("m1_s", [128, 4, 512], BF16)
        m2_s = pa("m2_s", [128, 4, 256], BF16)
        d1_s = pa("d1_s", [128, 256], BF16)
        d2_s = pa("d2_s", [128, 2, 256], BF16)
        bq_s = pa("bq_s", [128, 2, 1], FP32)
        bk_s = pa("bk_s", [128, 2, 1], FP32)
        bm_s = pa("bm_s", [128, 2, 1], FP32)
        m1b_s = pa("m1b_s", [128, 4, 1], FP32)
        m1g_s = pa("m1g_s", [128, 4, 1], FP32)
        m1be_s = pa("m1be_s", [128, 4, 1], FP32)
        m2b_s = pa("m2b_s", [128, 2, 1], FP32)
        d1b_s = pa("d1b_s", [128, 2, 1], FP32)
        d1g_s = pa("d1g_s", [128, 2, 1], FP32)
        d1be_s = pa("d1be_s", [128, 2, 1], FP32)
        d2b_s = pa("d2b_s", [128, 2, 1], FP32)
        bns = pa("bns", [128, 8, 6], FP32)
        par = pa("par", [128, 2, 4], FP32)
        tot = pa("tot", [128, 2, 4], FP32)
        parf = pa("parf", [128, 4, 4], FP32)
        totf = pa("totf", [128, 4, 4], FP32)
        pars = pa("pars", [128, 2, 2], FP32)
        tots = pa("tots", [128, 2, 2], FP32)
        dlt = pa("dlt", [128, 2, 1], FP32)
        dsq = pa("dsq", [128, 2, 1], FP32)
        res = pa("res", [1, 1], FP32)
        sv = pa("sv", [128, 200], FP32)   # scalar scratch, allocator below

        _svc = [0]

        def scol(n=1):
            b = _svc[0]
            _svc[0] += n
            assert _svc[0] <= 200
            return [sv[:, b + i:b + i + 1] for i in range(n)]

        dma = nc.sync.dma_start

        # ---------------- weights / biases ----------------
        for g in range(2):
            r = slice(g * 128, (g + 1) * 128)
            dma(wq_s[:, g, :], wqT[r, :])
            dma(wk_s[:, g, :], wkT[r, :])
            dma(wv_s[:, g, :], wvT[r, :])
            dma(d2_s[:, g, :], d2T[r, :])
            dma(bq_s[:, g, :], bqv[r, :])
            dma(bk_s[:, g, :], bkv[r, :])
            dma(bm_s[:, g, :], bmv[r, :])
            dma(m2b_s[:, g, :], m2b[r, :])
            dma(d1b_s[:, g, :], d1b[r, :])
            dma(d1g_s[:, g, :], d1g[r, :])
            dma(d1be_s[:, g, :], d1be[r, :])
            dma(d2b_s[:, g, :], d2b[r, :])
        dma(d1_s[:, :], d1T[:, :])
        for h in range(H):
            dma(wm_r[:, h, :], wmT[h * 64:(h + 1) * 64, :])
        for g in range(4):
            r = slice(g * 128, (g + 1) * 128)
            dma(m1_s[:, g, :], m1T[r, :])
            dma(m2_s[:, g, :], m2T[r, :])
            dma(m1b_s[:, g, :], m1b[r, :])
            dma(m1g_s[:, g, :], m1g[r, :])
            dma(m1be_s[:, g, :], m1be[r, :])
        nc.gpsimd.memset(ones[:, :], 1.0)

        def inv_std(var_ap, eps):
            t, s0, r0, s1, inv = scol(5)
            nc.vector.tensor_scalar_add(t, var_ap, float(eps))
            nc.scalar.activation(s0, t, AF.Sqrt)
            nc.vector.reciprocal(r0, s0)
            nc.vector.tensor_mul(r0, t, r0)
            nc.vector.tensor_add(r0, r0, s0)
            nc.vector.tensor_scalar_mul(s1, r0, 0.5)
            nc.vector.reciprocal(inv, s1)
            return inv

        # ------------- standardize p_src/p_tar (per row-group stream) -------------
        with tc.tile_pool(name="pin", bufs=2) as Pin:
            for nm, srcT, ownT, dst_bf, own_f32, own_b16 in (
                ("s", psT, ops, xs_bf, xs_own, xs_own_bf),
                ("t", ptT, opt_, xt_bf, xt_own, xt_own_bf),
            ):
                for g in range(2):
                    pbuf = Pin.tile([128, 8, 512], FP32, tag="pbuf",
                                    name=f"pbuf_{nm}{g}")
                    obuf = Pin.tile([128, 512], FP32, tag="obuf",
                                    name=f"obuf_{nm}{g}")
                    r = slice(g * 128, (g + 1) * 128)
                    dma(pbuf[:, :, :], srcT[r, :].rearrange("p (c f) -> p c f", f=512))
                    dma(obuf[:, :], ownT[r, :])
                    for c in range(8):
                        nc.vector.bn_stats(bns[:, c, :], pbuf[:, c, :])
                    ag2 = pa(f"ag_{nm}{g}", [128, 2], FP32)
                    nc.vector.bn_aggr(ag2[:, :], bns[:, :, :])
                    inv = inv_std(ag2[:, 1:2], 0.0)
                    (nb,) = scol(1)
                    nc.vector.tensor_mul(nb, ag2[:, 0:1], inv)
                    nc.vector.tensor_scalar_mul(nb, nb, -1.0)
                    nc.scalar.activation(dst_bf[:, g, :, :], pbuf[:, :, :],
                                         AF.Identity, bias=nb, scale=inv)
                    nc.scalar.activation(own_f32[:, g, :], obuf[:, :],
                                         AF.Identity, bias=nb, scale=inv)
                    nc.vector.tensor_copy(own_b16[:, g, :], own_f32[:, g, :])

            # ------------- dis stats + own shard scale + d1 conv -------------
            for nm, srcT, ownT, scdst in (
                ("ds", dsT, ods, sc_ds),
                ("dt", dtT, odt, sc_dt),
            ):
                dbuf = Pin.tile([128, 8, 512], FP32, tag="pbuf", name=f"dbuf_{nm}")
                obuf = Pin.tile([128, 512], FP32, tag="obuf", name=f"obuf_{nm}")
                dma(dbuf[:, :, :], srcT[:, :].rearrange("p (c f) -> p c f", f=512))
                dma(obuf[:, :], ownT[:, :])
                for c in range(8):
                    nc.vector.bn_stats(bns[:, c, :], dbuf[:, c, :])
                ag2 = pa(f"ag_{nm}", [128, 2], FP32)
                nc.vector.bn_aggr(ag2[:, :], bns[:, :, :])
                inv = inv_std(ag2[:, 1:2], 0.0)
                (nb,) = scol(1)
                nc.vector.tensor_mul(nb, ag2[:, 0:1], inv)
                nc.vector.tensor_scalar_mul(nb, nb, -1.0)
                nc.scalar.activation(scdst[:, :], obuf[:, :],
                                     AF.Identity, bias=nb, scale=inv)

        for src, dst in ((sc_ds, x1d_s), (sc_dt, x1d_t)):
            for og in range(2):
                mp = Psc.tile([128, 2, 512], FP32, tag="sc", name="mp_d1")
                nc.tensor.matmul(mp[:, 0, :], d1_s[:, og * 128:(og + 1) * 128],
                                 src[:, :], start=True, stop=True)
                nc.vector.tensor_scalar_add(dst[:, og, :], mp[:, 0, :],
                                            d1b_s[:, og, :])
        # partial BN stats for both d1 outputs -> one AllReduce
        sqd = pa("sqd", [128, 2, 512], FP32)
        for i, x1 in enumerate((x1d_s, x1d_t)):
            nc.vector.reduce_sum(par[:, :, 2 * i], x1[:, :, :], axis=AX.X)
            nc.scalar.activation(sqd[:, :, :], x1[:, :, :], AF.Square)
            nc.vector.reduce_sum(par[:, :, 2 * i + 1], sqd[:, :, :], axis=AX.X)
        ar_in = Dram.tile([D, 4], FP32, name="ar_in")
        ar_out = Dram.tile([D, 4], FP32, name="ar_out", addr_space="Shared")
        for g in range(2):
            dma(ar_in[g * 128:(g + 1) * 128, :], par[:, g, :])
        nc.gpsimd.collective_compute(
            "AllReduce", ALU.add, replica_groups=RG,
            ins=[ar_in.opt()], outs=[ar_out.opt()])

        # --- persistB: attention-era tensors ---
        PB = st.enter_context(tc.tile_pool(name="persistB", bufs=1))

        def pb(name, shape, dt, tag=None):
            return PB.tile(shape, dt, name=name, tag=tag or name)

        VT = pb("VT", [128, MT, H * HB], BF16)
        Kb = pb("Kb", [128, 2, 8, 512], BF16)
        ds_bf = pb("ds_bf", [128, 2, 8, 512], BF16)
        dt_bf = pb("dt_bf", [128, 2, 8, 512], BF16)
        d0f = pb("d0f", [128, 2, 8, 512], BF16)
        for h in range(H):
            nc.gpsimd.memset(VT[:, :, h * HB + HD], 1.0)

        def attention(tag, q_own_bf, k_src, v_src, merge_f32, merge_b16):
            # Q projection (+bias)
            qp = Psc.tile([128, 2, 512], FP32, tag="sc", name=f"qp_{tag}")
            for og in range(2):
                for cg in range(2):
                    nc.tensor.matmul(qp[:, og, :],
                                     wq_s[:, cg, og * 128:(og + 1) * 128],
                                     q_own_bf[:, cg, :],
                                     start=(cg == 0), stop=(cg == 1))
            for og in range(2):
                nc.scalar.activation(Qb[:, og, :], qp[:, og, :], AF.Identity,
                                     bias=bq_s[:, og, :])
            # K projection (+bias), full N
            for og in range(2):
                for c in range(8):
                    kp = Psc.tile([128, 2, 512], FP32, tag="sc", name=f"kp_{tag}")
                    for cg in range(2):
                        nc.tensor.matmul(kp[:, 0, :],
                                         wk_s[:, cg, og * 128:(og + 1) * 128],
                                         k_src[:, cg, c, :],
                                         start=(cg == 0), stop=(cg == 1))
                    nc.vector.tensor_scalar_add(Kb[:, og, c, :], kp[:, 0, :],
                                                bk_s[:, og, :])
            # V^T projection (keys on partitions), no bias (folded into bm)
            for m in range(MT):
                c, f0 = divmod(m * 128, 512)
                vp = Psc.tile([128, 2, 512], FP32, tag="sc", name=f"vp_{tag}")
                for cg in range(2):
                    nc.tensor.matmul(vp[:, 0, 0:256],
                                     v_src[:, cg, c, f0:f0 + 128],
                                     wv_s[:, cg, :],
                                     start=(cg == 0), stop=(cg == 1))
                for h in range(H):
                    nc.vector.tensor_copy(VT[:, m, h * HB:h * HB + HD],
                                          vp[:, 0, h * HD:(h + 1) * HD])
            # streaming attention per head, PV pipelined one group behind
            for h in range(H):
                hg, hp = h // 2, (h % 2) * 64
                op = Pout.tile([65, 512], FP32, tag="out", name=f"op_{tag}{h}")
                prev = None
                for g in range(16):
                    sc = Psc.tile([128, 2, 512], FP32, tag="sc", name=f"sc_{tag}")
                    for j in range(2):
                        m = g * 2 + j
                        c, f0 = divmod(m * 128, 512)
                        nc.tensor.matmul(sc[:, j, :],
                                         Kb[hp:hp + 64, hg, c, f0:f0 + 128],
                                         Qb[hp:hp + 64, hg, :],
                                         start=True, stop=True)
                    pr = Ppr.tile([128, 2, 512], BF16, tag="pr", name=f"pr_{tag}")
                    nc.scalar.activation(pr[:, :, :], sc[:, :, :], AF.Exp,
                                         scale=0.125)
                    if prev is not None:
                        for j in range(2):
                            m = prev[0] * 2 + j
                            nc.tensor.matmul(
                                op[:, :], VT[:, m, h * HB:(h + 1) * HB],
                                prev[1][:, j, :], start=(m == 0), stop=False)
                    prev = (g, pr)
                for j in range(2):
                    m = prev[0] * 2 + j
                    nc.tensor.matmul(op[:, :], VT[:, m, h * HB:(h + 1) * HB],
                                     prev[1][:, j, :], start=False,
                                     stop=(m == MT - 1))
                # evacuate raw numerator (bf16); denominator: psum row 64 ->
                # sbuf row 64 (ScalarE) -> partition 0 (DMA) -> reciprocal at
                # base 0 (custom DVE op misbehaves at base 64) -> PE broadcast
                # -> in-place normalize
                nc.vector.tensor_copy(An[:, h, :], op[0:64, :])
                dnm = Ppr.tile([65, 512], FP32, tag="dnm", bufs=2,
                               name=f"dnm_{tag}{h}")
                nc.scalar.activation(dnm[64:65, :], op[64:65, :], AF.Copy)
                dma(dnm[0:1, :], dnm[64:65, :])
                rc = Ppr.tile([1, 512], FP32, tag="rc", bufs=2,
                              name=f"rc_{tag}{h}")
                nc.vector.reciprocal_approx_fast(rc[0:1, :], dnm[0:1, :])
                bc = Pbc.tile([64, 512], FP32, tag="bc", name=f"bc_{tag}{h}")
                nc.tensor.matmul(bc[:, :], ones[0:1, 0:64], rc[0:1, :],
                                 start=True, stop=True)
                nc.vector.tensor_mul(An[:, h, :], An[:, h, :], bc[:, :])
            # merge: accumulate per head (K=64), + bm_eff at evacuation
            mg = Psc.tile([128, 2, 512], FP32, tag="sc", name=f"mg_{tag}")
            for og in range(2):
                for h in range(H):
                    nc.tensor.matmul(mg[:, og, :],
                                     wm_r[:, h, og * 128:(og + 1) * 128],
                                     An[:, h, :],
                                     start=(h == 0), stop=(h == 3))
            for og in range(2):
                if merge_f32 is not None:
                    nc.scalar.activation(merge_f32[:, og, :], mg[:, og, :],
                                         AF.Identity, bias=bm_s[:, og, :])
                    if merge_b16 is not None:
                        nc.vector.tensor_copy(merge_b16[:, og, :],
                                              merge_f32[:, og, :])
                else:
                    nc.scalar.activation(merge_b16[:, og, :], mg[:, og, :],
                                         AF.Identity, bias=bm_s[:, og, :])

        # ---------------- round 1a (dis AllReduce completes underneath) ----------
        ag_in = Dram.tile([2 * D, NQ], BF16, name="ag_in")
        ag_out = Dram.tile([NC * 2 * D, NQ], BF16, name="ag_out",
                           addr_space="Shared")
        attention("r1a", xs_own_bf, xt_bf, xt_bf, None, d_bf)
        mkdbg(dma, "dbg_xs_own", xs_own[:, :, :], (128, 2, 512), FP32)
        mkdbg(dma, "dbg_xs_bf", xs_bf[:, :, :, :], (128, 2, 8, 512), BF16)
        mkdbg(dma, "dbg_qb", Qb[:, :, :], (128, 2, 512), BF16)
        mkdbg(dma, "dbg_kb", Kb[:, :, :, :], (128, 2, 8, 512), BF16)
        mkdbg(dma, "dbg_vt", VT[:, :, :], (128, MT, H * HB), BF16)
        mkdbg(dma, "dbg_an", An[:, :, :], (64, 4, 512), BF16)
        mkdbg(dma, "dbg_dbf", d_bf[:, :, :], (128, 2, 512), BF16)
        for g in range(2):
            dma(ag_in[g * 128:(g + 1) * 128, :], d_bf[:, g, :])

        # ---- dis BN apply + relu + d2 (own shard) + AllGather ds/dt ----
        for g in range(2):
            dma(tot[:, g, :], ar_out[g * 128:(g + 1) * 128, :])
        for i, (x1, dst) in enumerate(((x1d_s, ds_own_bf), (x1d_t, dt_own_bf))):
            for og in range(2):
                mu, va, msq, a_, b_ = scol(5)
                nc.vector.tensor_scalar_mul(mu, tot[:, og, 2 * i:2 * i + 1],
                                            1.0 / N)
                nc.vector.tensor_scalar_mul(va, tot[:, og, 2 * i + 1:2 * i + 2],
                                            1.0 / N)
                nc.vector.tensor_mul(msq, mu, mu)
                nc.vector.tensor_sub(va, va, msq)
                inv = inv_std(va, EPS)
                nc.vector.tensor_mul(a_, d1g_s[:, og, :], inv)
                nc.vector.tensor_mul(b_, mu, a_)
                nc.vector.tensor_scalar_mul(b_, b_, -1.0)
                nc.vector.tensor_add(b_, b_, d1be_s[:, og, :])
                nc.scalar.activation(r_d[:, og, :], x1[:, og, :], AF.Relu,
                                     bias=b_, scale=a_)
            for og in range(2):
                mp = Psc.tile([128, 2, 512], FP32, tag="sc", name="mp_d2")
                for cg in range(2):
                    nc.tensor.matmul(mp[:, 0, :],
                                     d2_s[:, cg, og * 128:(og + 1) * 128],
                                     r_d[:, cg, :], start=(cg == 0), stop=(cg == 1))
                nc.vector.tensor_scalar_add(dst[:, og, :], mp[:, 0, :],
                                            d2b_s[:, og, :])
        mkdbg(dma, "dbg_x1d", x1d_s[:, :, :], (128, 2, 512), FP32)
        mkdbg(dma, "dbg_tot", tot[:, :, :], (128, 2, 4), FP32)
        mkdbg(dma, "dbg_dsown", ds_own_bf[:, :, :], (128, 2, 512), BF16)
        agd_in = Dram.tile([2 * D, NQ], BF16, name="agd_in")
        agd_out = Dram.tile([NC * 2 * D, NQ], BF16, name="agd_out",
                            addr_space="Shared")
        for g in range(2):
            dma(agd_in[g * 128:(g + 1) * 128, :], ds_own_bf[:, g, :])
            dma(agd_in[256 + g * 128:256 + (g + 1) * 128, :], dt_own_bf[:, g, :])
        nc.gpsimd.collective_compute(
            "AllGather", ALU.bypass, replica_groups=RG,
            ins=[agd_in.opt()], outs=[agd_out.opt()])

        # ---------------- round 1b (ds/dt AllGather completes underneath) --------
        attention("r1b", xt_own_bf, xs_bf, xs_bf, None, d1own_bf)
        for g in range(2):
            dma(ag_in[256 + g * 128:256 + (g + 1) * 128, :], d1own_bf[:, g, :])
        nc.gpsimd.collective_compute(
            "AllGather", ALU.bypass, replica_groups=RG,
            ins=[ag_in.opt()], outs=[ag_out.opt()])

        # gather ds/dt full
        for r in range(NC):
            for g in range(2):
                b0 = r * 2 * D
                dma(ds_bf[:, g, r, :], agd_out[b0 + g * 128:b0 + (g + 1) * 128, :])
                dma(dt_bf[:, g, r, :],
                    agd_out[b0 + 256 + g * 128:b0 + 256 + (g + 1) * 128, :])
        # gather deltas + gating: d0f = delta0*xt ; xt_bf <- delta1*xs (pt_tmp)
        for r in range(NC):
            b0 = r * 2 * D
            for g in range(2):
                dma(d0f[:, g, r, :], ag_out[b0 + g * 128:b0 + (g + 1) * 128, :])
        for g in range(2):
            nc.vector.tensor_mul(d0f[:, g, :, :], d0f[:, g, :, :],
                                 xt_bf[:, g, :, :])
        for r in range(NC):
            b0 = r * 2 * D
            for g in range(2):
                dma(xt_bf[:, g, r, :],
                    ag_out[b0 + 256 + g * 128:b0 + (g + 1) * 128 + 256, :])
        for g in range(2):
            nc.vector.tensor_mul(xt_bf[:, g, :, :], xt_bf[:, g, :, :],
                                 xs_bf[:, g, :, :])

        mkdbg(dma, "dbg_dsbf", ds_bf[:, :, :, :], (128, 2, 8, 512), BF16)
        mkdbg(dma, "dbg_d0f", d0f[:, :, :, :], (128, 2, 8, 512), BF16)
        mkdbg(dma, "dbg_pt", xt_bf[:, :, :, :], (128, 2, 8, 512), BF16)

        # ---------------- round 2 ----------------
        attention("r2a", ds_own_bf, dt_bf, d0f, dp0_f, dp0_bf)
        mkdbg(dma, "dbg_dp0", dp0_f[:, :, :], (128, 2, 512), FP32)
        attention("r2b", dt_own_bf, ds_bf, xt_bf, dp1_f, dp1_bf)
        mkdbg(dma, "dbg_dp1", dp1_f[:, :, :], (128, 2, 512), FP32)

        # ---------------- final mlp (sharded) + MMD ----------------
        x1_s = PB.tile([128, 4, 512], FP32, name="x1_s", tag="d0f")
        x1_t = PB.tile([128, 4, 512], FP32, name="x1_t", tag="Kb")
        sq = PB.tile([128, 4, 512], FP32, name="sq", tag="VT")
        for i, (xo, dp, x1) in enumerate(((xs_own_bf, dp0_bf, x1_s),
                                          (xt_own_bf, dp1_bf, x1_t))):
            rhs = [xo[:, 0, :], xo[:, 1, :], dp[:, 0, :], dp[:, 1, :]]
            for og in range(4):
                mp = Psc.tile([128, 2, 512], FP32, tag="sc", name="mp_m1")
                for cg in range(4):
                    nc.tensor.matmul(mp[:, 0, :],
                                     m1_s[:, cg, og * 128:(og + 1) * 128],
                                     rhs[cg], start=(cg == 0), stop=(cg == 3))
                nc.vector.tensor_scalar_add(x1[:, og, :], mp[:, 0, :],
                                            m1b_s[:, og, :])
            nc.vector.reduce_sum(parf[:, :, 2 * i], x1[:, :, :], axis=AX.X)
            nc.scalar.activation(sq[:, :, :], x1[:, :, :], AF.Square)
            nc.vector.reduce_sum(parf[:, :, 2 * i + 1], sq[:, :, :], axis=AX.X)
        arf_in = Dram.tile([2 * D, 4], FP32, name="arf_in")
        arf_out = Dram.tile([2 * D, 4], FP32, name="arf_out", addr_space="Shared")
        for g in range(4):
            dma(arf_in[g * 128:(g + 1) * 128, :], parf[:, g, :])
        nc.gpsimd.collective_compute(
            "AllReduce", ALU.add, replica_groups=RG,
            ins=[arf_in.opt()], outs=[arf_out.opt()])
        for g in range(4):
            dma(totf[:, g, :], arf_out[g * 128:(g + 1) * 128, :])

        mkdbg(dma, "dbg_x1s", x1_s[:, :, :], (128, 4, 512), FP32)
        mkdbg(dma, "dbg_totf", totf[:, :, :], (128, 4, 4), FP32)
        os_own = PB.tile([128, 2, 512], FP32, name="os_own", tag="ds_bf")
        ot_own = PB.tile([128, 2, 512], FP32, name="ot_own", tag="dt_bf")
        r_f = PB.tile([128, 4, 512], BF16, name="r_f", tag="VT")
        for i, (x1, xo, oo) in enumerate(((x1_s, xs_own, os_own),
                                          (x1_t, xt_own, ot_own))):
            for og in range(4):
                mu, va, msq, a_, b_ = scol(5)
                nc.vector.tensor_scalar_mul(mu, totf[:, og, 2 * i:2 * i + 1],
                                            1.0 / N)
                nc.vector.tensor_scalar_mul(va, totf[:, og, 2 * i + 1:2 * i + 2],
                                            1.0 / N)
                nc.vector.tensor_mul(msq, mu, mu)
                nc.vector.tensor_sub(va, va, msq)
                inv = inv_std(va, EPS)
                nc.vector.tensor_mul(a_, m1g_s[:, og, :], inv)
                nc.vector.tensor_mul(b_, mu, a_)
                nc.vector.tensor_scalar_mul(b_, b_, -1.0)
                nc.vector.tensor_add(b_, b_, m1be_s[:, og, :])
                nc.scalar.activation(r_f[:, og, :], x1[:, og, :], AF.Relu,
                                     bias=b_, scale=a_)
            for og in range(2):
                mp = Psc.tile([128, 2, 512], FP32, tag="sc", name="mp_m2")
                for cg in range(4):
                    nc.tensor.matmul(mp[:, 0, :],
                                     m2_s[:, cg, og * 128:(og + 1) * 128],
                                     r_f[:, cg, :], start=(cg == 0), stop=(cg == 3))
                nc.vector.scalar_tensor_tensor(
                    oo[:, og, :], mp[:, 0, :], m2b_s[:, og, :], xo[:, og, :],
                    op0=ALU.add, op1=ALU.add)

        mkdbg(dma, "dbg_os", os_own[:, :, :], (128, 2, 512), FP32)
        mkdbg(dma, "dbg_ot", ot_own[:, :, :], (128, 2, 512), FP32)
        nc.vector.reduce_sum(pars[:, :, 0], os_own[:, :, :], axis=AX.X)
        nc.vector.reduce_sum(pars[:, :, 1], ot_own[:, :, :], axis=AX.X)
        ars_in = Dram.tile([D, 2], FP32, name="ars_in")
        ars_out = Dram.tile([D, 2], FP32, name="ars_out", addr_space="Shared")
        for g in range(2):
            dma(ars_in[g * 128:(g + 1) * 128, :], pars[:, g, :])
        nc.gpsimd.collective_compute(
            "AllReduce", ALU.add, replica_groups=RG,
            ins=[ars_in.opt()], outs=[ars_out.opt()])
        for g in range(2):
            dma(tots[:, g, :], ars_out[g * 128:(g + 1) * 128, :])
        mkdbg(dma, "dbg_tots", tots[:, :, :], (128, 2, 2), FP32)
        for g in range(2):
            nc.vector.tensor_sub(dlt[:, g, :], tots[:, g, 0:1], tots[:, g, 1:2])
        nc.vector.tensor_scalar_mul(dlt[:, :, :], dlt[:, :, :], 1.0 / N)
        nc.scalar.activation(dsq[:, :, :], dlt[:, :, :], AF.Square)
        dot = Pbc.tile([65, 512], FP32, tag="bc", name="dot")
        for g in range(2):
            nc.tensor.matmul(dot[0:1, 0:1], dsq[:, g, :], ones[:, 0:1],
                             start=(g == 0), stop=(g == 1))
        nc.vector.tensor_copy(res[:, :], dot[0:1, 0:1])
        dma(out_dram[:, :], res[:, :])

        st.close()

    nc.compile()
    return nc


# head permutation: new row i = h*64+d  <- old channel d*4+h
_PERM = np.array([d * H + h for h in range(H) for d in range(HD)])


def _prep_inputs(inputs):
    bf16 = ml_dtypes.bfloat16
    f32 = np.float32

    def C(x, dt=f32):
        return np.ascontiguousarray(np.asarray(x), dtype=dt)

    p_src = C(inputs["p_src"])[0]
    p_tar = C(inputs["p_tar"])[0]
    dis_src = C(inputs["dis_src"])[0]
    dis_tar = C(inputs["dis_tar"])[0]
    aq_w = C(inputs["aq_w"]); ak_w = C(inputs["ak_w"])
    av_w = C(inputs["av_w"]); am_w = C(inputs["am_w"])
    shared = {
        "psT": C(p_src.T), "ptT": C(p_tar.T),
        "dsT": C(dis_src.T), "dtT": C(dis_tar.T),
        "wqT": C(aq_w[_PERM, :].T, bf16),
        "wkT": C(ak_w[_PERM, :].T, bf16),
        "wvT": C(av_w[_PERM, :].T, bf16),
        "wmT": C(am_w[:, _PERM].T, bf16),
        "bq": C(inputs["aq_b"])[_PERM].reshape(D, 1).copy(),
        "bk": C(inputs["ak_b"])[_PERM].reshape(D, 1).copy(),
        "bm": (C(inputs["am_b"]) + am_w @ C(inputs["av_b"])).reshape(D, 1),
        "m1T": C(C(inputs["m1_w"]).T, bf16),
        "m1b": C(inputs["m1_b"]).reshape(2 * D, 1),
        "m1g": C(inputs["m1_g"]).reshape(2 * D, 1),
        "m1be": C(inputs["m1_be"]).reshape(2 * D, 1),
        "m2T": C(C(inputs["m2_w"]).T, bf16),
        "m2b": C(inputs["m2_b"]).reshape(D, 1),
        "d1T": C(C(inputs["d1_w"]).T, bf16),
        "d1b": C(inputs["d1_b"]).reshape(D, 1),
        "d1g": C(inputs["d1_g"]).reshape(D, 1),
        "d1be": C(inputs["d1_be"]).reshape(D, 1),
        "d2T": C(C(inputs["d2_w"]).T, bf16),
        "d2b": C(inputs["d2_b"]).reshape(D, 1),
    }
    in_maps = []
    for c in range(NC):
        sl = slice(c * NQ, (c + 1) * NQ)
        m = dict(shared)
        m["ops"] = C(p_src[sl, :].T)
        m["opt"] = C(p_tar[sl, :].T)
        m["ods"] = C(dis_src[sl, :].T)
        m["odt"] = C(dis_tar[sl, :].T)
        in_maps.append(m)
    return in_maps


def kernel(**inputs):
    from concourse.bass_utils import run_bass_kernel_spmd

    if "nc" not in _CACHE:
        _CACHE["nc"] = _build_program()
    nc = _CACHE["nc"]
    in_maps = _prep_inputs(inputs)
    res = run_bass_kernel_spmd(nc, in_maps, core_ids=list(range(NC)))
    return np.asarray(res.results[0]["out"], np.float32).reshape(())
